# revision 6
# baseline (speedup 1.0000x reference)
"""Trainium2 Bass kernel for nn_AttentionOperation (sparse_attention).

Computation (per the reference):
    sim  = QK^T                  [N,H,L,L]
    sim  = BN_heads(sim)         (stats over b,l,m per head)
    attn = softmax(sim, -1)
    rv   = attn @ V^T            [N,H,C,L] -> [N, H*C, L]
    rv   = BN_channels(rv)       (stats over b,l per channel)
    out  = gelu_exact(rv)

Sharding: one head per NeuronCore (H=8, n_cores=8).  Both BatchNorms are
then fully core-local, so there is no communication.

Key device-side structure (v2):
  * BN1 mean/bias shift cancels inside the softmax, so only
    g = w_h * rsqrt(var + eps) is needed.  var comes from tiny Gram
    matmuls on an fp8 copy of Q/K (stats-only path; ~0.2% var error),
    which halves the largest input DMA.
  * PSUM layout: a single 6-bank sim ring [128, 3072] that 512-col QK
    matmuls rotate through, plus one 2-bank tag for gram/AV tiles.
    The ring lets each Exp activation cover 1536 columns (22 ACT
    instructions instead of 32), cutting per-instruction overhead on
    the bottleneck engine.
  * Exp outputs stream into one [128, 32768] fp16 SBUF arena that AV
    matmuls read 512-col slices from.
  * softmax denominator comes free from a ones-row appended to V^T;
    reciprocal on DVE, den copy + partition-broadcast on Pool (GpSimd),
    keeping ACT exclusively on exp/gelu.
  * matmul operands fp16 (full-rate on PE; fp32 would be 4x slower).
    PE instruction stream is ordered to stay continuously busy so the
    clock ramps to the full 2.4 GHz p-state.
  * rsqrt is a DVE-only quake-seed Newton iteration; BN2 affine is
    folded into the Gelu activation's scale/bias operands.
"""

import numpy as np

N, H, D, L = 4, 8, 64, 1024
C = 64
NCH = L // 128          # m-chunks of 128
EPS = 1e-3
CNT = float(N * L * L)  # elements per head for sim BN stats
NPOS = 4 * NCH * 2      # 64 512-col QK positions
NEXP = (NPOS * 512 + 1535) // 1536  # 22 exp instructions
RING = 3072             # sim ring columns (6 PSUM banks)

_CACHE = {}


def _build_nc():
    import concourse.bacc as bacc
    import concourse.tile as tile
    import concourse.mybir as mybir

    f32 = mybir.dt.float32
    f16 = mybir.dt.float16
    f8 = mybir.dt.float8e4
    i32 = mybir.dt.int32
    AF = mybir.ActivationFunctionType
    ALU = mybir.AluOpType

    nc = bacc.Bacc("TRN2", target_bir_lowering=False, debug=False)

    qk2_d = nc.dram_tensor("qk2", [128, 2, 2, L], f16, kind="ExternalInput")
    kqo_d = nc.dram_tensor("kqo", [128, N, NCH, 129], f8,
                           kind="ExternalInput")
    vo_d = nc.dram_tensor("vo", [128, N, NCH, 65], f16, kind="ExternalInput")
    id_d = nc.dram_tensor("ident", [128, 64], f16, kind="ExternalInput")
    wsv_d = nc.dram_tensor("wsv", [64, 3], f32, kind="ExternalInput")
    out_d = nc.dram_tensor("out", [N, 64, L], f32, kind="ExternalOutput")

    with tile.TileContext(nc) as tc:
        with (
            tc.tile_pool(name="cst", bufs=1) as cst,
            tc.tile_pool(name="sm", bufs=1) as sm,
            tc.tile_pool(name="ps", bufs=1, space="PSUM") as psp,
        ):
            # ---- input DMAs.  Queue order matters: the gram (kqo) and
            # QK (qk2 pair0) inputs gate the whole pipeline, so they go
            # first; vo is not needed until the first AV (~5us later) and
            # is issued from DVE after the gram copies.
            wsv_sb = cst.tile([64, 3], f32)
            nc.sync.dma_start(wsv_sb[:], wsv_d.ap())
            id_sb = cst.tile([128, 64], f16)
            nc.sync.dma_start(id_sb[:], id_d.ap())
            kqo_sb = cst.tile([128, N, NCH, 129], f8)
            for b in range(N):
                nc.sync.dma_start(kqo_sb[:, b], kqo_d.ap()[:, b])
            qk2_sb = cst.tile([128, 2, 2, L], f16)
            for p in range(2):
                nc.scalar.dma_start(qk2_sb[:, p], qk2_d.ap()[:, p])

            onesc = cst.tile([1, 1], f32)
            nc.vector.memset(onesc[:], 1.0)
            ones64 = cst.tile([64, 1], f32)
            nc.vector.memset(ones64[:], 1.0)
            # dummy exp so the ACT exp-table load happens off the critical
            # path (otherwise it lands right before the first real exp)
            warm_sb = sm.tile([1, 1], f32, tag="warm", bufs=1)
            nc.scalar.activation(warm_sb[:], onesc[:], AF.Exp)

            # ---- BN1 stats: one stacked gram matmul per (batch, chunk) ----
            # G[b] = [k|q|1]^T [k|q|1]:  KK = G[0:64,0:64],
            # QQ = G[64:128,64:128], ksum = G[0:64,128], qsum = G[64:128,128]
            kk_sb = sm.tile([64, N, 129], f32, tag="kk", bufs=1)
            qsrc_sb = sm.tile([128, N, 65], f16, tag="gk", bufs=1)
            for b in range(N):
                gps = psp.tile([128, 129], f32, tag="av", bufs=2,
                               name=f"gram_ps_{b}")
                for c in range(NCH):
                    nc.tensor.matmul(
                        gps[:], kqo_sb[:, b, c, 0:128], kqo_sb[:, b, c, :],
                        start=(c == 0), stop=(c == NCH - 1))
                nc.scalar.copy(kk_sb[:, b, :], gps[0:64, :])
                nc.vector.tensor_copy(qsrc_sb[64:128, b, :],
                                      gps[64:128, 64:129])
            # ONE identity matmul realigns all four QQ blocks onto
            # partitions 0-63 (fp16: full-rate)
            qq_ps = psp.tile([64, N, 65], f32, tag="av", bufs=2)
            nc.tensor.matmul(qq_ps[:], id_sb[64:128, :],
                             qsrc_sb[64:128, :, :], start=True, stop=True)
            qq_sb = sm.tile([64, N, 65], f32, tag="gq", bufs=1)
            nc.vector.tensor_copy(qq_sb[:], qq_ps[:])

            qsp = sm.tile([64, 2], f32, tag="qs", bufs=1)
            sprod = sm.tile([64, N], f32, tag="sprod", bufs=1)
            pscr = sm.tile([64, N, 64], f32, tag="pscr", bufs=1)
            nc.vector.tensor_tensor(
                out=pscr[:], in0=kk_sb[:, :, 0:64], in1=qq_sb[:, :, 0:64],
                op=ALU.mult)
            nc.vector.tensor_reduce(
                out=qsp[:, 0:1], in_=pscr[:],
                axis=mybir.AxisListType.XY, op=ALU.add)
            nc.vector.tensor_tensor(
                out=sprod[:], in0=kk_sb[:, :, 128], in1=qq_sb[:, :, 64],
                op=ALU.mult)
            nc.vector.tensor_reduce(
                out=qsp[:, 1:2], in_=sprod[:],
                axis=mybir.AxisListType.X, op=ALU.add)

            # partition-sum via PE: out [1,2] = [sum(sim^2), sum(sim)]
            scps = psp.tile([1, 2], f32, tag="av", bufs=2)
            nc.tensor.matmul(scps[:], ones64[:], qsp[:], start=True,
                             stop=True)
            qs2 = sm.tile([1, 2], f32, tag="qs2", bufs=1)
            nc.vector.tensor_copy(qs2[:], scps[:])

            # DVE-only rsqrt(x + eps): quake seed + 2 Newton iterations.
            def dve_rsqrt(dst_ap, x_ap, p, pref):
                xe = sm.tile([p, 1], f32, tag=f"{pref}xe", bufs=1,
                             name=f"{pref}_xe")
                nc.vector.tensor_scalar_add(xe[:], x_ap, EPS)
                sh = sm.tile([p, 1], i32, tag=f"{pref}sh", bufs=1,
                             name=f"{pref}_sh")
                nc.vector.tensor_scalar(
                    out=sh[:], in0=xe[:].bitcast(i32), scalar1=1,
                    scalar2=None, op0=ALU.arith_shift_right)
                magic = sm.tile([p, 1], i32, tag=f"{pref}mg", bufs=1,
                                name=f"{pref}_mg")
                nc.vector.memset(magic[:], 0x5F3759DF)
                y = sm.tile([p, 1], f32, tag=f"{pref}y", bufs=1,
                            name=f"{pref}_y")
                nc.vector.tensor_tensor(out=y[:].bitcast(i32), in0=magic[:],
                                        in1=sh[:], op=ALU.subtract)
                t = sm.tile([p, 1], f32, tag=f"{pref}t", bufs=1,
                            name=f"{pref}_t")
                n_it = 2  # seed err 3.4% -> 1.7e-3 -> 4e-6: plenty here
                for it in range(n_it):
                    nc.vector.tensor_tensor(out=t[:], in0=y[:], in1=y[:],
                                            op=ALU.mult)
                    nc.vector.scalar_tensor_tensor(
                        out=t[:], in0=t[:], scalar=-0.5, in1=xe[:],
                        op0=ALU.mult, op1=ALU.mult)
                    nc.vector.scalar_tensor_tensor(
                        out=(dst_ap if it == n_it - 1 else y[:]), in0=t[:],
                        scalar=1.5, in1=y[:], op0=ALU.add, op1=ALU.mult)

            # var = E[x^2] - E[x]^2 ; g = w_h * rsqrt(var + eps)
            eq_t = sm.tile([1, 1], f32, tag="sc1", bufs=1)
            nc.vector.tensor_scalar_mul(eq_t[:], qs2[:, 0:1], 1.0 / CNT)
            m2_t = sm.tile([1, 1], f32, tag="sc2", bufs=1)
            nc.vector.scalar_tensor_tensor(
                out=m2_t[:], in0=qs2[:, 1:2], scalar=1.0 / (CNT * CNT),
                in1=qs2[:, 1:2], op0=ALU.mult, op1=ALU.mult)
            var_t = sm.tile([1, 1], f32, tag="sc3", bufs=1)
            nc.vector.tensor_tensor(out=var_t[:], in0=eq_t[:], in1=m2_t[:],
                                    op=ALU.subtract)
            rs_t = sm.tile([1, 1], f32, tag="sc5", bufs=1)
            dve_rsqrt(rs_t[:], var_t[:], 1, "g")
            g_t = sm.tile([1, 1], f32, tag="sc6", bufs=1)
            nc.vector.tensor_tensor(out=g_t[:], in0=rs_t[:],
                                    in1=wsv_sb[0:1, 2:3], op=ALU.mult)
            # broadcast g to all 128 partitions (gpsimd)
            g128 = cst.tile([128, 1], f32)
            nc.gpsimd.partition_broadcast(g128[:], g_t[:], channels=128)

            # vo arrives late on purpose (Pool-issued after the g chain so
            # its descriptors don't steal queue bandwidth from kqo/qk2)
            vo_sb = cst.tile([128, N, NCH, 65], f16)
            for hb in range(2):
                nc.gpsimd.dma_start(vo_sb[:, 2 * hb:2 * hb + 2],
                                    vo_d.ap()[:, 2 * hb:2 * hb + 2])

            # ---- main attention pipeline ----
            # sim ring: ONE 6-bank PSUM tile; 512-col QK outputs rotate
            # through it (sub-tile deps give RAW/WAR sync automatically).
            ring = psp.tile([128, RING], f32, tag="ring", bufs=1)
            # exp arena: all 32 exp'd chunk-tiles side by side in SBUF
            arena = cst.tile([128, NPOS * 512], f16)
            # rv / gelu-out arenas (big free dims -> wide epilogue ops)
            rv_ar = cst.tile([64, N * L], f32)
            out_ar = cst.tile([64, N * L], f32)
            stats = cst.tile([64, 2 * N, 6], f32)

            def emit_qk(p):
                b, r = divmod(p, 16)
                ch, half = divmod(r, 2)
                pair, b_in = divmod(b, 2)
                r0 = 64 * b_in
                rc = 512 * (p % 6)
                nc.tensor.matmul(
                    ring[:, rc:rc + 512],
                    qk2_sb[r0:r0 + 64, pair, 1, 128 * ch:128 * (ch + 1)],
                    qk2_sb[r0:r0 + 64, pair, 0, 512 * half:512 * (half + 1)],
                    start=True, stop=True)

            def emit_exp(e):
                w = min(1536, NPOS * 512 - 1536 * e)
                rc = 1536 * (e % 2)
                nc.scalar.activation(arena[:, 1536 * e:1536 * e + w],
                                     ring[:, rc:rc + w], AF.Exp,
                                     scale=g128[:, 0:1])

            def emit_av_and_epilogue(b):
                for half in range(2):
                    av_ps = psp.tile([65, 512], f32, tag="av", bufs=2,
                                     name=f"av_ps_{b}_{half}")
                    for ch in range(NCH):
                        ac = 8192 * b + 1024 * ch + 512 * half
                        nc.tensor.matmul(
                            av_ps[:], vo_sb[:, b, ch, :],
                            arena[:, ac:ac + 512],
                            start=(ch == 0), stop=(ch == NCH - 1))
                    # softmax denominator -> reciprocal -> broadcast -> rv
                    den_sb = sm.tile([1, 512], f32, tag="den", bufs=4,
                                     name=f"den_{b}_{half}")
                    nc.vector.tensor_copy(den_sb[:], av_ps[64:65, :])
                    rcp_sb = sm.tile([1, 512], f32, tag="rcp", bufs=4,
                                     name=f"rcp_{b}_{half}")
                    nc.vector.reciprocal_approx_fast(
                        out=rcp_sb[:], in_=den_sb[:])
                    rbc_sb = sm.tile([64, 512], f32, tag="rbc", bufs=4,
                                     name=f"rbc_{b}_{half}")
                    nc.gpsimd.partition_broadcast(
                        rbc_sb[:], rcp_sb[:], channels=64)
                    sl = slice(1024 * b + 512 * half,
                               1024 * b + 512 * half + 512)
                    nc.vector.tensor_tensor(
                        out=rv_ar[:, sl], in0=av_ps[0:64, :],
                        in1=rbc_sb[:], op=ALU.mult)
                    nc.vector.bn_stats(stats[:, 2 * b + half, :],
                                       rv_ar[:, sl])

            av_due = {5: 0, 10: 1, 15: 2, NEXP - 1: 3}
            for e in range(NEXP):
                for p in range(3 * e, min(3 * e + 3, NPOS)):
                    emit_qk(p)
                emit_exp(e)
                if e == NEXP - 1:
                    # hoist the gelu table load: pinned right after the
                    # last exp so it overlaps the AV/BN2 epilogue instead
                    # of sitting in front of the gelus
                    nc.scalar.activation(warm_sb[:], onesc[:], AF.Gelu)
                if e in av_due:
                    emit_av_and_epilogue(av_due[e])

            # ---- BN2 + gelu epilogue (affine folded into Gelu) ----
            mv = sm.tile([64, 2], f32, tag="mv", bufs=1)
            nc.vector.bn_aggr(mv[:], stats[:])
            rsv = sm.tile([64, 1], f32, tag="rsv", bufs=1)
            dve_rsqrt(rsv[:], mv[:, 1:2], 64, "v")
            scale_c = sm.tile([64, 1], f32, tag="sclc", bufs=1)
            nc.vector.tensor_tensor(out=scale_c[:], in0=rsv[:],
                                    in1=wsv_sb[:, 0:1], op=ALU.mult)
            mt = sm.tile([64, 1], f32, tag="mt", bufs=1)
            nc.vector.tensor_tensor(out=mt[:], in0=mv[:, 0:1], in1=scale_c[:],
                                    op=ALU.mult)
            bias_c = sm.tile([64, 1], f32, tag="bsc", bufs=1)
            nc.vector.tensor_tensor(out=bias_c[:], in0=wsv_sb[:, 1:2],
                                    in1=mt[:], op=ALU.subtract)

            for b in range(N):
                nc.scalar.activation(out_ar[:, L * b:L * (b + 1)],
                                     rv_ar[:, L * b:L * (b + 1)], AF.Gelu,
                                     bias=bias_c[:, 0:1],
                                     scale=scale_c[:, 0:1])
                # Pool-issued output DMA: 25ns sequencer cost vs 565 on SP
                nc.gpsimd.dma_start(out_d.ap()[b],
                                    out_ar[:, L * b:L * (b + 1)])

    nc.compile()
    return nc


def _host_inputs(query, key, value, bn_sim_weight, bn_sim_bias,
                 bn_val_weight, bn_val_bias, h):
    """Build the per-core (per-head) input map, with host-side layout prep."""
    import ml_dtypes
    f32 = np.float32
    f16 = np.float16
    f8 = ml_dtypes.float8_e4m3fn
    qh = np.asarray(query[:, h], dtype=f32)   # [4, 64, 1024]
    kh = np.asarray(key[:, h], dtype=f32)
    vh = np.asarray(value[:, h], dtype=f32)

    def pack_pairs(x):
        # [4, 64, L] -> [128, 2, L]; row b_in*64+d, slot (pair, l)
        return (x.reshape(2, 2, 64, L).transpose(1, 2, 0, 3)
                .reshape(128, 2, L).astype(f16))

    qk2 = np.empty((128, 2, 2, L), dtype=f16)
    qk2[:, :, 0, :] = pack_pairs(qh)
    qk2[:, :, 1, :] = pack_pairs(kh)

    def chunked_t(x):
        # [4, 64, L] -> [128(m), 4(b), 8(chunk), 64]
        return x.transpose(2, 0, 1).reshape(NCH, 128, N, 64).transpose(
            1, 2, 0, 3)

    kq = np.empty((128, N, NCH, 129), dtype=f8)
    kq[..., 0:64] = chunked_t(kh).astype(f8)
    kq[..., 64:128] = chunked_t(qh).astype(f8)
    kq[..., 128] = 1.0

    vo = np.empty((128, N, NCH, 65), dtype=f16)
    vo[..., :64] = chunked_t(vh).astype(f16)
    vo[..., 64] = 1.0

    ident = np.zeros((128, 64), dtype=f16)
    ident[64:128] = np.eye(64, dtype=f16)

    wsv = np.zeros((64, 3), dtype=f32)
    wsv[:, 0] = np.asarray(bn_val_weight[h * 64:(h + 1) * 64], dtype=f32)
    wsv[:, 1] = np.asarray(bn_val_bias[h * 64:(h + 1) * 64], dtype=f32)
    wsv[0, 2] = np.float32(bn_sim_weight[h])

    return {
        "ident": ident,
        "qk2": np.ascontiguousarray(qk2),
        "kqo": np.ascontiguousarray(kq),
        "vo": np.ascontiguousarray(vo),
        "wsv": wsv,
    }


def get_nc():
    if "nc" not in _CACHE:
        _CACHE["nc"] = _build_nc()
    return _CACHE["nc"]


def make_in_maps(**inputs):
    return [_host_inputs(
        inputs["query"], inputs["key"], inputs["value"],
        inputs["bn_sim_weight"], inputs["bn_sim_bias"],
        inputs["bn_val_weight"], inputs["bn_val_bias"], h) for h in range(H)]


def kernel(**inputs):
    from concourse.bass_utils import run_bass_kernel_spmd

    nc = get_nc()
    in_maps = make_in_maps(**inputs)
    res = run_bass_kernel_spmd(nc, in_maps, core_ids=list(range(H)))
    outs = [np.asarray(res.results[i]["out"]) for i in range(H)]
    return np.ascontiguousarray(
        np.concatenate(outs, axis=1).astype(np.float32))
